# revision 19
# baseline (speedup 1.0000x reference)
"""Trainium2 Bass kernel for nn_DecGreenNet_product_CP3.

Reference computation:
    lhs  = tanh(input @ Wx1 + bx1) @ Wx2 + bx2          # [B, 512]
    s_i  = sum_n sin(pi*eq*qx_n) * mlp_i(qx_n)           # [8,16] per branch
    rhs  = einsum('bx,dx,fx->bdf', s_a, s_c, s_e)        # [512]
    out  = lhs @ rhs                                     # [B]

Algebraic restructuring (validated to ~2e-3 rel err):
    out[b] = tanh(input[b] @ Wx1 + bx1) @ (Wx2 @ rhs) + bx2 @ rhs
    z      = h1q^T @ y ; s = W2^T z + (sum y) * b2   per quad branch
collapsing the dominant [B,512]x[512,512] GEMM into a matvec.

Sharding: batch B split 8 ways (8192 rows/core); the quadrature branch is
REPLICATED on every core (no collective -- the 8-core AllReduce costs
~40-60us of barrier/mesh machinery plus launch-skew exposure, far more
than the ~25us of replicated quad tanh).

Engine plan per core (ScalarE is the bottleneck, ~59us busy):
  ScalarE: 16 quad tanh [128,1536] + 21 main tanh [128,2048/1024]
  PE: row-tiled small-K GEMMs (tile_position) for quad (K=2) and main L1
      (K=4); y-stationary matvecs for z; dot matvecs for the output.
  DVE: sin() minimax poly for y, einsum, final (dot+c)*2^36 rows.
"""

import numpy as np

import concourse.bacc as bacc
import concourse.bass as bass
import concourse.mybir as mybir
import concourse.tile as tile
from concourse.bass_utils import run_bass_kernel_spmd

F32 = mybir.dt.float32
F16 = mybir.dt.float16
AF = mybir.ActivationFunctionType
ALU = mybir.AluOpType

NCORES = 8
B, DIN, H = 65536, 3, 512
N, HQ = 8192, 128
BL = B // NCORES          # 8192 batch rows per core
NT = N // 128             # 64 node tiles per branch
TT = 3 * NT               # 192 flat node tiles
QSG = 16                  # quad supergroups, 12 tiles each
CH = 512                  # batch chunk (columns per dot)
NCH = BL // CH            # 16 chunks
NUNITS = NCH * 4          # 64 (chunk, h-tile) units

# fp16 scaling: w values are ~1e10-1e11; scale into fp16 range (exact pow2)
RC_SCALE = 2.0 ** -36     # applied to rhs_vec before the fp16 w-matmuls
OUT_SCALE = 2.0 ** 36     # undo in the final output pass

# minimax odd polynomial for sin(t), t in [0, pi]: sin(t)=t*P(t^2), err<2e-5
SIN_C = (0.999984590176674, -0.16663258473611252, 8.312385898666645e-03,
         -1.9316230946716391e-04, 2.1732361127812407e-06)

_CACHED_NC = None

import os
_STAGE = os.environ.get("K_STAGE", "full")  # y | z | s | r16 | full


def _qtile(t):
    """flat node tile t -> (supergroup S, strip g, wave G, column offset)."""
    S, j = divmod(t, 12)
    g, G = divmod(j, 4)
    return S, g, G, g * 512 + G * 128


# main-phase supergroup unit lists: A=4 units, B=2 units, alternating
def _main_sgs():
    sgs = []
    u = 0
    a = True
    while u < NUNITS:
        n = 4 if a else 2
        n = min(n, NUNITS - u)
        sgs.append(list(range(u, u + n)))
        u += n
        a = not a
    return sgs


MAIN_SGS = _main_sgs()


def _build():
    nc = bacc.Bacc("TRN2", target_bir_lowering=False, debug=False,
                   num_devices=NCORES)

    xT4 = nc.dram_tensor("xT4", [16, BL], F16, kind="ExternalInput").ap()
    wx1a4 = nc.dram_tensor("wx1a4", [16, H], F16, kind="ExternalInput").ap()
    wx2t = nc.dram_tensor("wx2tb", [64, 4096], F16, kind="ExternalInput").ap()
    bx2r = nc.dram_tensor("bx2rb", [64, 128], F16, kind="ExternalInput").ap()
    qxa4 = nc.dram_tensor("qxa4", [6, N], F16, kind="ExternalInput").ap()
    wqa4 = nc.dram_tensor("wqa4", [6, 384], F16, kind="ExternalInput").ap()
    qxc = nc.dram_tensor("qxc", [128, TT], F32, kind="ExternalInput").ap()
    wq2 = nc.dram_tensor("wq2", [HQ, 3 * HQ], F32, kind="ExternalInput").ap()
    bq2r = nc.dram_tensor("bq2r", [1, 3 * HQ], F32, kind="ExternalInput").ap()
    out_d = nc.dram_tensor("out", [BL], F32, kind="ExternalOutput").ap()

    global _APS
    _APS = (xT4, wx1a4, wx2t, bx2r, qxa4, wqa4, qxc, wq2, bq2r, out_d)
    with tile.TileContext(nc) as tc:
        _body(nc, tc)
    nc.compile()
    return nc


def _body(nc, tc):
    xT4, wx1a4, wx2t, bx2r, qxa4, wqa4, qxc, wq2, bq2r, out_d = _APS
    with (
        tc.tile_pool(name="const", bufs=1) as constp,
        tc.tile_pool(name="qsb", bufs=1) as qsb,
        tc.tile_pool(name="h1p", bufs=8) as h1p,
        tc.tile_pool(name="mainsb", bufs=1) as mainsb,
        tc.tile_pool(name="esb", bufs=2) as esb,
        tc.tile_pool(name="hidA", bufs=8) as hidA,
        tc.tile_pool(name="hidB", bufs=8) as hidB,
        tc.tile_pool(name="orowp", bufs=3) as orowp,
        tc.tile_pool(name="dram", bufs=1, space="DRAM") as dram,
        tc.tile_pool(name="smallp", bufs=1, space="PSUM") as smallp,
    ):
        # -------- ACT table preload: tiny tanh before any DMA lands ------
        warm = constp.tile([128, 1], F32)
        nc.vector.memset(warm, 0.0)
        warm16 = constp.tile([128, 1], F16)
        nc.scalar.activation(out=warm16, in_=warm, func=AF.Tanh)

        ones128 = constp.tile([128, 1], F32)
        nc.vector.memset(ones128, 1.0)

        # ---------------- input DMAs (quad gemm operands first) ---------
        # quad gemm operands at row strips {32g, 32g+1}, g=0..2
        qxa_sb = qsb.tile([66, N], F16, tag="qxa")
        wqa_sb = qsb.tile([66, 384], F16, tag="wqa")
        for g in range(3):
            nc.sync.dma_start(out=qxa_sb[32 * g:32 * g + 2, :],
                              in_=qxa4[2 * g:2 * g + 2, :])
            nc.sync.dma_start(out=wqa_sb[32 * g:32 * g + 2, :],
                              in_=wqa4[2 * g:2 * g + 2, :])
        # qxc (pre-scaled by pi*eq on host) on the scalar queue, which is
        # otherwise idle until the late wx2t load
        qxc_sb = qsb.tile([128, TT], F32, tag="qxc")
        nc.scalar.dma_start(out=qxc_sb, in_=qxc)
        # big late-needed loads are deferred into the quad loop so they
        # don't clog the DMA engines ahead of the quad-critical inputs
        wq2_sb = qsb.tile([HQ, 3 * HQ], F32, tag="wq2")
        bq2r_sb = qsb.tile([1, 3 * HQ], F32, tag="bq2r")
        xT_sb = mainsb.tile([100, BL], F16, tag="xT")
        wx1_sb = mainsb.tile([100, H], F16, tag="wx1")
        wx2t_sb = mainsb.tile([64, 4096], F16, tag="wx2t")
        bx2r_sb = mainsb.tile([64, 128], F16, tag="bx2r")

        def emit_late_dmas():
            # real dependency: this dummy DMA reads the first quad ACT's
            # output, so everything behind it on the in-order sync queue
            # fires only after the quad-critical inputs are long resident
            gateD = dram.tile([1, 8], F16, tag="gateD")
            nc.sync.dma_start(out=gateD, in_=h1q_tiles[0][0:1, 0:8])
            for g in range(4):
                nc.sync.dma_start(out=xT_sb[32 * g:32 * g + 4, :],
                                  in_=xT4[4 * g:4 * g + 4, :])
                nc.sync.dma_start(out=wx1_sb[32 * g:32 * g + 4, :],
                                  in_=wx1a4[4 * g:4 * g + 4, :])
            nc.sync.dma_start(out=wq2_sb, in_=wq2)
            nc.sync.dma_start(out=bq2r_sb, in_=bq2r)
            nc.sync.dma_start(out=wx2t_sb, in_=wx2t)
            nc.sync.dma_start(out=bx2r_sb, in_=bx2r)

        # ------- y = sin(t), t = pi*eq*qx pre-scaled on host in qxc -----
        tq = qxc_sb
        t2 = qsb.tile([128, TT], F32, tag="t2")
        nc.vector.tensor_tensor(out=t2, in0=tq, in1=tq, op=ALU.mult)
        pp = qsb.tile([128, TT], F32, tag="pp")
        c1, c3, c5, c7, c9 = [float(v) for v in SIN_C]
        nc.vector.tensor_scalar(out=pp, in0=t2, scalar1=c9, scalar2=c7,
                                op0=ALU.mult, op1=ALU.add)
        for cof in (c5, c3, c1):
            nc.vector.tensor_tensor(out=pp, in0=pp, in1=t2, op=ALU.mult)
            nc.vector.tensor_scalar_add(pp, pp, cof)
        y_sb = qsb.tile([128, TT], F16, tag="ysb")
        nc.vector.tensor_tensor(out=y_sb, in0=pp, in1=tq, op=ALU.mult)

        def qdump(ap2d):
            p, c = ap2d.shape[0], ap2d.shape[1]
            nc.sync.dma_start(
                out=out_d[0:p * c].rearrange("(p c) -> p c", c=c),
                in_=ap2d)

        if _STAGE == "y":
            y32 = qsb.tile([128, 4], F32, tag="y32")
            nc.vector.tensor_copy(out=y32, in_=y_sb[:, 0:4])
            qdump(y32)
            return

        # z accumulator psum: cols 0..383 = z per branch, 384..386 = sy
        z_ps = smallp.tile([1, 387], F32, tag="sm0")

        # sy = sum_n y[n] per branch (DVE reduce + ones matvec)
        ysum = []
        for br in range(3):
            t = qsb.tile([128, 1], F32, tag=f"ysum{br}")
            nc.vector.tensor_reduce(
                out=t, in_=y_sb[:, br * NT:(br + 1) * NT],
                axis=mybir.AxisListType.X, op=ALU.add)
            ysum.append(t)

        # ---------------- quad phase: fills + tanh + z ------------------
        h1q_tiles = [None] * QSG

        def qfill(S):
            pre = qprep.tile([128, 1536], F32, tag="qpre")
            for jj in range(12):
                G, g = divmod(jj, 3)
                j = g * 4 + G
                t = 12 * S + j
                br = t // NT
                nc.tensor.matmul(
                    pre[:, g * 512 + G * 128: g * 512 + (G + 1) * 128],
                    lhsT=qxa_sb[32 * g:32 * g + 2,
                                (t // 12) * 512 + G * 128:
                                (t // 12) * 512 + (G + 1) * 128],
                    rhs=wqa_sb[32 * g:32 * g + 2,
                               128 * br:128 * (br + 1)],
                    start=True, stop=True, tile_position=(32 * g, 0))
            h1 = h1p.tile([128, 1536], F16, tag="h1q")
            nc.scalar.activation(out=h1, in_=pre, func=AF.Tanh)
            h1q_tiles[S] = h1

        zcur = [0]

        def emit_z(limit, count):
            # emit up to `count` z matvecs for tiles < limit (acts done)
            while zcur[0] < limit and count > 0:
                t = zcur[0]
                S, j = divmod(t, 12)
                g, G = divmod(j, 4)
                br, jb = divmod(t, NT)
                nc.tensor.matmul(
                    z_ps[0:1, 128 * br:128 * (br + 1)],
                    lhsT=y_sb[:, t:t + 1],
                    rhs=h1q_tiles[S][:, g * 512 + G * 128:
                                     g * 512 + (G + 1) * 128],
                    start=(jb == 0), stop=(jb == NT - 1),
                    skip_group_check=True)
                zcur[0] += 1
                count -= 1

        # z -> s -> einsum -> w chain (emitted once z matvecs complete)
        ein = {}

        def emit_chain():
            z_sb = qsb.tile([1, 387], F32, tag="zsb")
            nc.vector.tensor_copy(out=z_sb, in_=z_ps)
            if _STAGE == "z":
                qdump(z_sb)
                return
            # transpose z to [128, 3]: PE transpose-mode, in_^T @ [[1.0]]
            zT_ps = smallp.tile([128, 3], F32, tag="sm1")
            for br in range(3):
                nc.tensor.transpose(
                    out=zT_ps[:, br:br + 1],
                    in_=z_sb[0:1, br * 128:(br + 1) * 128],
                    identity=ones128[0:1, 0:1])
            zT_sb = qsb.tile([128, 3], F32, tag="zT")
            nc.vector.tensor_copy(out=zT_sb, in_=zT_ps)
            # s = W2^T z + sy * b2 per branch
            s_ps = smallp.tile([128, 3], F32, tag="sm1")
            for br in range(3):
                nc.tensor.matmul(
                    s_ps[:, br:br + 1],
                    lhsT=wq2_sb[:, br * HQ:(br + 1) * HQ],
                    rhs=zT_sb[:, br:br + 1], start=True, stop=False)
                nc.tensor.matmul(
                    s_ps[:, br:br + 1],
                    lhsT=bq2r_sb[0:1, br * HQ:(br + 1) * HQ],
                    rhs=z_sb[0:1, 384 + br:385 + br], start=False, stop=True)
            s_sb = qsb.tile([128, 3], F32, tag="ssb")
            nc.vector.tensor_copy(out=s_sb, in_=s_ps)
            if _STAGE == "s":
                qdump(s_sb[:, 0:3])
                return

            # sT[16 x, (br, 8 b)] via dram bounce (partition-split transpose)
            sD = dram.tile([16, 24], F32, tag="sD")
            nc.gpsimd.dma_start(
                out=sD.rearrange("x (c b) -> b x c", b=8), in_=s_sb)
            sT_sb = esb.tile([16, 24], F32, tag="sT")
            nc.gpsimd.dma_start(out=sT_sb, in_=sD)
            # E[x, d*8+f] = s_c[d,x] * s_e[f,x]
            sc_ap = sT_sb[:, 8:16]
            se_ap = sT_sb[:, 16:24]
            in0 = bass.AP(tensor=sc_ap.tensor, offset=sc_ap.offset,
                          ap=[sc_ap.ap[0], sc_ap.ap[1], [0, 8]])
            in1 = bass.AP(tensor=se_ap.tensor, offset=se_ap.offset,
                          ap=[se_ap.ap[0], [0, 8], se_ap.ap[1]])
            E_sb = esb.tile([16, 64], F32, tag="E")
            nc.vector.tensor_tensor(
                out=E_sb.rearrange("p (d f) -> p d f", f=8),
                in0=in0, in1=in1, op=ALU.mult)
            ein["E"] = E_sb
            ein["sT"] = sT_sb

        def emit_chain_b():
            E_sb, sT_sb = ein["E"], ein["sT"]
            # rhs_vec: out[bdf] = sum_x sT_a[x,b] * E[x,df] -> [64 df, 8 b]
            rhsp = smallp.tile([64, 8], F32, tag="sm0")
            nc.tensor.matmul(rhsp, lhsT=E_sb, rhs=sT_sb[:, 0:8],
                             start=True, stop=True)
            r16 = esb.tile([64, 8], F16, tag="r16")
            nc.vector.tensor_scalar_mul(r16, rhsp, float(RC_SCALE))
            if _STAGE == "r16":
                qdump(r16)
                return
            # w = Wx2 @ rhs_vec as [128, 4] (h = it*128+p)
            wps = smallp.tile([128, 4], F32, tag="sm1")
            for it in range(4):
                for b in range(8):
                    nc.tensor.matmul(
                        wps[:, it:it + 1],
                        lhsT=wx2t_sb[:, b * 512 + it * 128:
                                     b * 512 + (it + 1) * 128],
                        rhs=r16[:, b:b + 1],
                        start=(b == 0), stop=(b == 7))
            w_sb = esb.tile([128, 4], F16, tag="wsb")
            nc.vector.tensor_copy(out=w_sb, in_=wps)
            # c (scalar, scaled by RC_SCALE) replicated over 16 partitions
            c16p = smallp.tile([16, 1], F32, tag="sm0")
            for b in range(8):
                nc.tensor.matmul(
                    c16p, lhsT=bx2r_sb[:, b * 16:(b + 1) * 16],
                    rhs=r16[:, b:b + 1],
                    start=(b == 0), stop=(b == 7))
            c16_sb = esb.tile([16, 1], F32, tag="c16")
            nc.vector.tensor_copy(out=c16_sb, in_=c16p)
            ein["w"] = w_sb
            ein["c16"] = c16_sb

        with tc.tile_pool(name="qprep", bufs=2, space="PSUM") as qprep:
            qfill(0)
            qfill(1)
            # sy matvecs early (ones stationary; separate psum groups)
            for br in range(3):
                nc.tensor.matmul(
                    z_ps[0:1, 384 + br:385 + br], lhsT=ysum[br],
                    rhs=ones128, start=True, stop=True,
                    skip_group_check=True)
            emit_late_dmas()
            for S in range(2, QSG):
                qfill(S)
                if S >= 3:
                    emit_z(12 * (S - 1), 12)

        if _STAGE in ("z", "s", "r16"):
            emit_z(TT, TT)
            emit_chain()
            if _STAGE == "r16":
                emit_chain_b()
            return

        # ---------------- main phase: L1 fills + tanh + dots ------------
        # unit u = (c, ht) = (u // 4, u % 4); hid slice map: u -> (tile, col)
        hid_slice = {}

        def mfill(k):
            units = MAIN_SGS[k]
            n = len(units)
            if n == 4:
                pre = mainA.tile([128, 2048], F32, tag="preA")
                hid = hidA.tile([128, 2048], F16, tag="hidA")
            else:
                pre = mainB.tile([128, 1024], F32, tag="preB")
                hid = hidB.tile([128, 1024], F16, tag="hidB")
            for g, u in enumerate(units):
                c, ht = divmod(u, 4)
                nc.tensor.matmul(
                    pre[:, g * 512:(g + 1) * 512],
                    lhsT=wx1_sb[32 * g:32 * g + 4,
                                ht * 128:(ht + 1) * 128],
                    rhs=xT_sb[32 * g:32 * g + 4, c * CH:(c + 1) * CH],
                    start=True, stop=True, tile_position=(32 * g, 0))
                hid_slice[u] = (hid, g * 512)
            nc.scalar.activation(out=hid, in_=pre, func=AF.Tanh)

        def emit_dot(c):
            op = smallp.tile([1, 512], F32, tag=f"sm{c % 2}", name=f"dot{c}")
            for ht in range(4):
                hid, col = hid_slice[4 * c + ht]
                nc.tensor.matmul(
                    op, lhsT=ein["w"][:, ht:ht + 1],
                    rhs=hid[:, col:col + 512],
                    start=(ht == 0), stop=(ht == 3))
            orow = orowp.tile([1, 512], F32, tag="orow")
            nc.vector.tensor_scalar(
                out=orow, in0=op, scalar1=ein["c16"][0:1, 0:1],
                scalar2=float(OUT_SCALE), op0=ALU.add, op1=ALU.mult)
            nc.sync.dma_start(
                out=out_d[c * CH:(c + 1) * CH].rearrange(
                    "(o b) -> o b", o=1),
                in_=orow)

        with (
            tc.tile_pool(name="mainA", bufs=1, space="PSUM") as mainA,
            tc.tile_pool(name="mainB", bufs=1, space="PSUM") as mainB,
        ):
            next_chunk = 0
            chain_k = [99]
            for k in range(len(MAIN_SGS)):
                mfill(k)
                # finish the z matvecs, then the s/einsum/w chain, then dots
                if zcur[0] < TT:
                    emit_z(TT, 12)
                    if zcur[0] >= TT:
                        emit_chain()
                        chain_k[0] = k
                elif "w" not in ein:
                    if k >= chain_k[0] + 2:
                        emit_chain_b()
                elif k >= 2:
                    covered = sum(len(MAIN_SGS[i]) for i in range(k - 1))
                    nd = 0
                    cap = 2 if k < 14 else 3
                    while (next_chunk + 1) * 4 <= covered and nd < cap:
                        emit_dot(next_chunk)
                        next_chunk += 1
                        nd += 1
            while next_chunk < NCH:
                emit_dot(next_chunk)
                next_chunk += 1


def _get_nc():
    global _CACHED_NC
    if _CACHED_NC is None:
        _CACHED_NC = _build()
    return _CACHED_NC


def _prep_in_maps(inputs):
    f = lambda k: np.ascontiguousarray(np.asarray(inputs[k], np.float32))
    inputx = f("input")
    eq = float(np.asarray(inputs["eq_param"]).reshape(-1)[0])
    Wx1, bx1 = f("Wx1"), f("bx1")
    Wx2, bx2 = f("Wx2"), f("bx2")

    # main L1 operands replicated at 4 row strips
    wx1a = np.concatenate([Wx1, bx1[None, :]], axis=0)       # [4, 512]
    wx1a4 = np.tile(wx1a, (4, 1)).astype(np.float16)         # [16, 512]
    # wx2tb[df, b*512+it*128+i] = Wx2T[b*64+df, it*128+i]
    wx2tb = np.ascontiguousarray(
        Wx2.T.reshape(8, 64, 4, 128).transpose(1, 0, 2, 3).reshape(64, 4096)
    ).astype(np.float16)
    # bx2rb[df, b*16+m] = bx2[b*64+df]
    bx2rb = np.ascontiguousarray(
        np.repeat(bx2.reshape(8, 64).T[:, :, None], 16, axis=2).reshape(64, 128)
    ).astype(np.float16)

    # quad operands (shared across cores; quad fully replicated)
    qs, w1r, b1r = [], [], []
    wq2 = np.empty((HQ, 3 * HQ), np.float32)
    bq2r = np.empty((1, 3 * HQ), np.float32)
    for br, (qk, w1k, b1k, w2k, b2k) in enumerate([
            ("quad_x0", "Wq01", "bq01", "Wq02", "bq02"),
            ("quad_x1", "Wq11", "bq11", "Wq12", "bq12"),
            ("quad_x2", "Wq21", "bq21", "Wq22", "bq22")]):
        qs.append(f(qk)[:, 0])
        w1r.append(f(w1k)[0])
        b1r.append(f(b1k))
        wq2[:, br * HQ:(br + 1) * HQ] = f(w2k)
        bq2r[0, br * HQ:(br + 1) * HQ] = f(b2k)

    # qxa4[2g + r, S*512 + G*128 + i]: strip g holds tiles 12S+4g+G
    qxa4 = np.empty((6, N), np.float32)
    qxc = np.empty((128, TT), np.float32)
    for t in range(TT):
        br, j = divmod(t, NT)
        nodes = qs[br][j * 128:(j + 1) * 128]
        S, g, G, _ = _qtile(t)
        qxa4[2 * g, S * 512 + G * 128: S * 512 + (G + 1) * 128] = nodes
        qxa4[2 * g + 1] = 1.0
        qxc[:, t] = nodes * (np.pi * eq)
    # wqa4[2g + r, 128*br + h]: r=0 -> W1 row, r=1 -> b1
    wqa4 = np.empty((6, 384), np.float32)
    for g in range(3):
        for br in range(3):
            wqa4[2 * g, 128 * br:128 * (br + 1)] = w1r[br]
            wqa4[2 * g + 1, 128 * br:128 * (br + 1)] = b1r[br]

    shared = dict(wx1a4=wx1a4, wx2tb=wx2tb, bx2rb=bx2rb,
                  qxa4=qxa4.astype(np.float16), wqa4=wqa4.astype(np.float16),
                  qxc=np.ascontiguousarray(qxc), wq2=wq2, bq2r=bq2r)

    in_maps = []
    ones_row = np.ones((1, BL), np.float32)
    for c in range(NCORES):
        ish = inputx[c * BL:(c + 1) * BL]                    # [8192, 3]
        xTm = np.concatenate([ish.T, ones_row], axis=0)      # [4, 8192]
        m = dict(shared)
        m["xT4"] = np.ascontiguousarray(
            np.tile(xTm, (4, 1))).astype(np.float16)         # [16, 8192]
        in_maps.append(m)
    return in_maps


def _run(inputs, **kw):
    nc = _get_nc()
    in_maps = _prep_in_maps(inputs)
    res = run_bass_kernel_spmd(nc, in_maps, list(range(NCORES)), **kw)
    out = np.concatenate([res.results[c]["out"].reshape(-1)
                          for c in range(NCORES)]).astype(np.float32)
    return out, res


def kernel(**inputs) -> np.ndarray:
    out, _ = _run(inputs)
    return out


def kernel_traced(**inputs):
    """Correctness + NTFF profile (exec_time_ns) in one run."""
    return _run(inputs, trace=True)


# revision 20
# speedup vs baseline: 1.1707x; 1.1707x over previous
"""Trainium2 Bass kernel for nn_DecGreenNet_product_CP3.

Reference computation:
    lhs  = tanh(input @ Wx1 + bx1) @ Wx2 + bx2          # [B, 512]
    s_i  = sum_n sin(pi*eq*qx_n) * mlp_i(qx_n)           # [8,16] per branch
    rhs  = einsum('bx,dx,fx->bdf', s_a, s_c, s_e)        # [512]
    out  = lhs @ rhs                                     # [B]

Algebraic restructuring (validated to ~2e-3 rel err):
    out[b] = tanh(input[b] @ Wx1 + bx1) @ (Wx2 @ rhs) + bx2 @ rhs
    z      = h1q^T @ y ; s = W2^T z + (sum y) * b2   per quad branch
collapsing the dominant [B,512]x[512,512] GEMM into a matvec.

Sharding: batch B split 8 ways (8192 rows/core); the quadrature branch is
REPLICATED on every core (no collective -- the 8-core AllReduce costs
~40-60us of barrier/mesh machinery plus launch-skew exposure, far more
than the ~25us of replicated quad tanh).

Engine plan per core (ScalarE is the bottleneck, ~59us busy):
  ScalarE: 16 quad tanh [128,1536] + 21 main tanh [128,2048/1024]
  PE: row-tiled small-K GEMMs (tile_position) for quad (K=2) and main L1
      (K=4); y-stationary matvecs for z; dot matvecs for the output.
  DVE: sin() minimax poly for y, einsum, final (dot+c)*2^36 rows.
"""

import numpy as np

import concourse.bacc as bacc
import concourse.bass as bass
import concourse.mybir as mybir
import concourse.tile as tile
from concourse.bass_utils import run_bass_kernel_spmd

F32 = mybir.dt.float32
F16 = mybir.dt.float16
AF = mybir.ActivationFunctionType
ALU = mybir.AluOpType

NCORES = 8
B, DIN, H = 65536, 3, 512
N, HQ = 8192, 128
BL = B // NCORES          # 8192 batch rows per core
NT = N // 128             # 64 node tiles per branch
TT = 3 * NT               # 192 flat node tiles
QSG = 16                  # quad supergroups, 12 tiles each
CH = 512                  # batch chunk (columns per dot)
NCH = BL // CH            # 16 chunks
NUNITS = NCH * 4          # 64 (chunk, h-tile) units

# fp16 scaling: w values are ~1e10-1e11; scale into fp16 range (exact pow2)
RC_SCALE = 2.0 ** -36     # applied to rhs_vec before the fp16 w-matmuls
OUT_SCALE = 2.0 ** 36     # undo in the final output pass

# minimax odd polynomial for sin(t), t in [0, pi]: sin(t)=t*P(t^2), err<2e-5
SIN_C = (0.999984590176674, -0.16663258473611252, 8.312385898666645e-03,
         -1.9316230946716391e-04, 2.1732361127812407e-06)

_CACHED_NC = None

import os
_STAGE = os.environ.get("K_STAGE", "full")  # y | z | s | r16 | full


def _qtile(t):
    """flat node tile t -> (supergroup S, strip g, wave G, column offset)."""
    S, j = divmod(t, 12)
    g, G = divmod(j, 4)
    return S, g, G, g * 512 + G * 128


# main-phase supergroup unit lists: A=4 units, B=2 units, alternating
def _main_sgs():
    sgs = []
    u = 0
    a = True
    while u < NUNITS:
        n = 4 if a else 2
        n = min(n, NUNITS - u)
        sgs.append(list(range(u, u + n)))
        u += n
        a = not a
    return sgs


MAIN_SGS = _main_sgs()


def _build():
    nc = bacc.Bacc("TRN2", target_bir_lowering=False, debug=False,
                   num_devices=NCORES)

    qxa4 = nc.dram_tensor("qxa4", [6, N + 384], F16, kind="ExternalInput").ap()
    qxc = nc.dram_tensor("qxc", [128, TT], F32, kind="ExternalInput").ap()
    xT1 = nc.dram_tensor("xT1", [4, BL], F16, kind="ExternalInput").ap()
    wx1a4 = nc.dram_tensor("wx1a4", [16, H], F16, kind="ExternalInput").ap()
    wx2t = nc.dram_tensor("wx2tb", [64, 4096], F16, kind="ExternalInput").ap()
    bx2r = nc.dram_tensor("bx2rb", [64, 128], F16, kind="ExternalInput").ap()
    wq2 = nc.dram_tensor("wq2", [HQ, 3 * HQ], F16, kind="ExternalInput").ap()
    bq2r = nc.dram_tensor("bq2r", [1, 3 * HQ], F16, kind="ExternalInput").ap()
    out_d = nc.dram_tensor("out", [BL], F32, kind="ExternalOutput").ap()

    global _APS
    _APS = (xT1, wx1a4, wx2t, bx2r, qxa4, qxc, wq2, bq2r, out_d)
    with tile.TileContext(nc) as tc:
        _body(nc, tc)
    nc.compile()
    return nc


def _body(nc, tc):
    xT1, wx1a4, wx2t, bx2r, qxa4, qxc, wq2, bq2r, out_d = _APS
    with (
        tc.tile_pool(name="const", bufs=1) as constp,
        tc.tile_pool(name="qsb", bufs=1) as qsb,
        tc.tile_pool(name="h1p", bufs=8) as h1p,
        tc.tile_pool(name="mainsb", bufs=1) as mainsb,
        tc.tile_pool(name="esb", bufs=2) as esb,
        tc.tile_pool(name="hidA", bufs=8) as hidA,
        tc.tile_pool(name="hidB", bufs=8) as hidB,
        tc.tile_pool(name="orowp", bufs=3) as orowp,
        tc.tile_pool(name="dram", bufs=1, space="DRAM") as dram,
        tc.tile_pool(name="smallp", bufs=1, space="PSUM") as smallp,
    ):
        # -------- ACT table preload: tiny tanh before any DMA lands ------
        warm = constp.tile([128, 1], F32)
        nc.vector.memset(warm, 0.0)
        warm16 = constp.tile([128, 1], F16)
        nc.scalar.activation(out=warm16, in_=warm, func=AF.Tanh)

        ones128 = constp.tile([128, 1], F32)
        nc.vector.memset(ones128, 1.0)

        # ---------------- input DMAs (quad gemm operands first) ---------
        # quad gemm operands at row strips {32g, 32g+1}, g=0..2; the wqa
        # weights ride in the last 384 columns of the same rows
        qxa_sb = qsb.tile([66, N + 384], F16, tag="qxa")
        for g in range(3):
            nc.sync.dma_start(out=qxa_sb[32 * g:32 * g + 2, :],
                              in_=qxa4[2 * g:2 * g + 2, :])
        # qxc (pre-scaled by pi*eq on host) on the scalar queue, which is
        # otherwise idle until the late wx2t load
        qxc_sb = qsb.tile([128, TT], F32, tag="qxc")
        nc.scalar.dma_start(out=qxc_sb, in_=qxc)
        # big late-needed loads are deferred into the quad loop so they
        # don't clog the DMA engines ahead of the quad-critical inputs
        wq2_sb = qsb.tile([HQ, 3 * HQ], F16, tag="wq2")
        bq2r_sb = qsb.tile([1, 3 * HQ], F16, tag="bq2r")
        xT_sb = mainsb.tile([100, BL], F16, tag="xT")
        wx1_sb = mainsb.tile([100, H], F16, tag="wx1")
        wx2t_sb = mainsb.tile([64, 4096], F16, tag="wx2t")
        bx2r_sb = mainsb.tile([64, 128], F16, tag="bx2r")

        def emit_late_dmas():
            # real dependency: this dummy DMA reads the first quad ACT's
            # output, so everything behind it on the in-order sync queue
            # fires only after the quad-critical inputs are long resident
            gateD = dram.tile([1, 8], F16, tag="gateD")
            nc.sync.dma_start(out=gateD, in_=h1q_tiles[0][0:1, 0:8])
            nc.sync.dma_start(out=xT_sb[0:4, :], in_=xT1)
            for g in range(1, 4):
                nc.sync.dma_start(out=xT_sb[32 * g:32 * g + 4, :],
                                  in_=xT_sb[0:4, :])
            for g in range(4):
                nc.sync.dma_start(out=wx1_sb[32 * g:32 * g + 4, :],
                                  in_=wx1a4[4 * g:4 * g + 4, :])
            nc.sync.dma_start(out=wq2_sb, in_=wq2)
            nc.sync.dma_start(out=bq2r_sb, in_=bq2r)
            nc.sync.dma_start(out=wx2t_sb, in_=wx2t)
            nc.sync.dma_start(out=bx2r_sb, in_=bx2r)

        # ------- y = sin(t), t = pi*eq*qx pre-scaled on host in qxc -----
        tq = qxc_sb
        t2 = qsb.tile([128, TT], F32, tag="t2")
        nc.vector.tensor_tensor(out=t2, in0=tq, in1=tq, op=ALU.mult)
        pp = qsb.tile([128, TT], F32, tag="pp")
        c1, c3, c5, c7, c9 = [float(v) for v in SIN_C]
        nc.vector.tensor_scalar(out=pp, in0=t2, scalar1=c9, scalar2=c7,
                                op0=ALU.mult, op1=ALU.add)
        for cof in (c5, c3, c1):
            nc.vector.tensor_tensor(out=pp, in0=pp, in1=t2, op=ALU.mult)
            nc.vector.tensor_scalar_add(pp, pp, cof)
        y_sb = qsb.tile([128, TT], F16, tag="ysb")
        nc.vector.tensor_tensor(out=y_sb, in0=pp, in1=tq, op=ALU.mult)

        def qdump(ap2d):
            p, c = ap2d.shape[0], ap2d.shape[1]
            nc.sync.dma_start(
                out=out_d[0:p * c].rearrange("(p c) -> p c", c=c),
                in_=ap2d)

        if _STAGE == "y":
            y32 = qsb.tile([128, 4], F32, tag="y32")
            nc.vector.tensor_copy(out=y32, in_=y_sb[:, 0:4])
            qdump(y32)
            return

        # z accumulator psum: cols 0..383 = z per branch, 384..386 = sy
        z_ps = smallp.tile([1, 387], F32, tag="sm0")

        # sy = sum_n y[n] per branch (DVE reduce + ones matvec)
        ysum = []
        for br in range(3):
            t = qsb.tile([128, 1], F32, tag=f"ysum{br}")
            nc.vector.tensor_reduce(
                out=t, in_=y_sb[:, br * NT:(br + 1) * NT],
                axis=mybir.AxisListType.X, op=ALU.add)
            ysum.append(t)

        # ---------------- quad phase: fills + tanh + z ------------------
        h1q_tiles = [None] * QSG

        def qfill(S):
            pre = qprep.tile([128, 1536], F32, tag="qpre")
            for jj in range(12):
                G, g = divmod(jj, 3)
                j = g * 4 + G
                t = 12 * S + j
                br = t // NT
                nc.tensor.matmul(
                    pre[:, g * 512 + G * 128: g * 512 + (G + 1) * 128],
                    lhsT=qxa_sb[32 * g:32 * g + 2,
                                (t // 12) * 512 + G * 128:
                                (t // 12) * 512 + (G + 1) * 128],
                    rhs=qxa_sb[32 * g:32 * g + 2,
                               N + 128 * br:N + 128 * (br + 1)],
                    start=True, stop=True, tile_position=(32 * g, 0))
            h1 = h1p.tile([128, 1536], F16, tag="h1q")
            nc.scalar.activation(out=h1, in_=pre, func=AF.Tanh)
            h1q_tiles[S] = h1

        zcur = [0]

        def emit_z(limit, count):
            # emit up to `count` z matvecs for tiles < limit (acts done)
            while zcur[0] < limit and count > 0:
                t = zcur[0]
                S, j = divmod(t, 12)
                g, G = divmod(j, 4)
                br, jb = divmod(t, NT)
                nc.tensor.matmul(
                    z_ps[0:1, 128 * br:128 * (br + 1)],
                    lhsT=y_sb[:, t:t + 1],
                    rhs=h1q_tiles[S][:, g * 512 + G * 128:
                                     g * 512 + (G + 1) * 128],
                    start=(jb == 0), stop=(jb == NT - 1),
                    skip_group_check=True)
                zcur[0] += 1
                count -= 1

        # z -> s -> einsum -> w chain (emitted once z matvecs complete)
        ein = {}

        def emit_chain():
            z_sb = qsb.tile([1, 387], F32, tag="zsb")
            nc.vector.tensor_copy(out=z_sb, in_=z_ps)
            if _STAGE == "z":
                qdump(z_sb)
                return
            # transpose z to [128, 3]: PE transpose-mode, in_^T @ [[1.0]]
            zT_ps = smallp.tile([128, 3], F32, tag="sm1")
            for br in range(3):
                nc.tensor.transpose(
                    out=zT_ps[:, br:br + 1],
                    in_=z_sb[0:1, br * 128:(br + 1) * 128],
                    identity=ones128[0:1, 0:1])
            zT_sb = qsb.tile([128, 3], F16, tag="zT")
            nc.vector.tensor_copy(out=zT_sb, in_=zT_ps)
            sy16 = qsb.tile([1, 3], F16, tag="sy16")
            nc.vector.tensor_copy(out=sy16, in_=z_sb[0:1, 384:387])
            # s = W2^T z + sy * b2 per branch
            s_ps = smallp.tile([128, 3], F32, tag="sm1")
            for br in range(3):
                nc.tensor.matmul(
                    s_ps[:, br:br + 1],
                    lhsT=wq2_sb[:, br * HQ:(br + 1) * HQ],
                    rhs=zT_sb[:, br:br + 1], start=True, stop=False)
                nc.tensor.matmul(
                    s_ps[:, br:br + 1],
                    lhsT=bq2r_sb[0:1, br * HQ:(br + 1) * HQ],
                    rhs=sy16[0:1, br:br + 1], start=False, stop=True)
            s_sb = qsb.tile([128, 3], F32, tag="ssb")
            nc.vector.tensor_copy(out=s_sb, in_=s_ps)
            if _STAGE == "s":
                qdump(s_sb[:, 0:3])
                return

            # sT[16 x, (br, 8 b)] via dram bounce (partition-split transpose)
            sD = dram.tile([16, 24], F32, tag="sD")
            nc.gpsimd.dma_start(
                out=sD.rearrange("x (c b) -> b x c", b=8), in_=s_sb)
            sT_sb = esb.tile([16, 24], F32, tag="sT")
            nc.gpsimd.dma_start(out=sT_sb, in_=sD)
            # E[x, d*8+f] = s_c[d,x] * s_e[f,x]
            sc_ap = sT_sb[:, 8:16]
            se_ap = sT_sb[:, 16:24]
            in0 = bass.AP(tensor=sc_ap.tensor, offset=sc_ap.offset,
                          ap=[sc_ap.ap[0], sc_ap.ap[1], [0, 8]])
            in1 = bass.AP(tensor=se_ap.tensor, offset=se_ap.offset,
                          ap=[se_ap.ap[0], [0, 8], se_ap.ap[1]])
            E_sb = esb.tile([16, 64], F32, tag="E")
            nc.vector.tensor_tensor(
                out=E_sb.rearrange("p (d f) -> p d f", f=8),
                in0=in0, in1=in1, op=ALU.mult)
            ein["E"] = E_sb
            ein["sT"] = sT_sb

        def emit_chain_b():
            E_sb, sT_sb = ein["E"], ein["sT"]
            # rhs_vec: out[bdf] = sum_x sT_a[x,b] * E[x,df] -> [64 df, 8 b]
            rhsp = smallp.tile([64, 8], F32, tag="sm0")
            nc.tensor.matmul(rhsp, lhsT=E_sb, rhs=sT_sb[:, 0:8],
                             start=True, stop=True)
            r16 = esb.tile([64, 8], F16, tag="r16")
            nc.vector.tensor_scalar_mul(r16, rhsp, float(RC_SCALE))
            if _STAGE == "r16":
                qdump(r16)
                return
            # w = Wx2 @ rhs_vec as [128, 4] (h = it*128+p)
            wps = smallp.tile([128, 4], F32, tag="sm1")
            for it in range(4):
                for b in range(8):
                    nc.tensor.matmul(
                        wps[:, it:it + 1],
                        lhsT=wx2t_sb[:, b * 512 + it * 128:
                                     b * 512 + (it + 1) * 128],
                        rhs=r16[:, b:b + 1],
                        start=(b == 0), stop=(b == 7))
            w_sb = esb.tile([128, 4], F16, tag="wsb")
            nc.vector.tensor_copy(out=w_sb, in_=wps)
            # c (scalar, scaled by RC_SCALE) replicated over 16 partitions
            c16p = smallp.tile([16, 1], F32, tag="sm0")
            for b in range(8):
                nc.tensor.matmul(
                    c16p, lhsT=bx2r_sb[:, b * 16:(b + 1) * 16],
                    rhs=r16[:, b:b + 1],
                    start=(b == 0), stop=(b == 7))
            c16_sb = esb.tile([16, 1], F32, tag="c16")
            nc.vector.tensor_copy(out=c16_sb, in_=c16p)
            ein["w"] = w_sb
            ein["c16"] = c16_sb

        with tc.tile_pool(name="qprep", bufs=2, space="PSUM") as qprep:
            qfill(0)
            qfill(1)
            # sy matvecs early (ones stationary; separate psum groups)
            for br in range(3):
                nc.tensor.matmul(
                    z_ps[0:1, 384 + br:385 + br], lhsT=ysum[br],
                    rhs=ones128, start=True, stop=True,
                    skip_group_check=True)
            emit_late_dmas()
            for S in range(2, QSG):
                qfill(S)
                if S >= 3:
                    emit_z(12 * (S - 1), 12)

        if _STAGE in ("z", "s", "r16"):
            emit_z(TT, TT)
            emit_chain()
            if _STAGE == "r16":
                emit_chain_b()
            return

        # ---------------- main phase: L1 fills + tanh + dots ------------
        # unit u = (c, ht) = (u // 4, u % 4); hid slice map: u -> (tile, col)
        hid_slice = {}

        def mfill(k):
            units = MAIN_SGS[k]
            n = len(units)
            if n == 4:
                pre = mainA.tile([128, 2048], F32, tag="preA")
                hid = hidA.tile([128, 2048], F16, tag="hidA")
            else:
                pre = mainB.tile([128, 1024], F32, tag="preB")
                hid = hidB.tile([128, 1024], F16, tag="hidB")
            for g, u in enumerate(units):
                c, ht = divmod(u, 4)
                nc.tensor.matmul(
                    pre[:, g * 512:(g + 1) * 512],
                    lhsT=wx1_sb[32 * g:32 * g + 4,
                                ht * 128:(ht + 1) * 128],
                    rhs=xT_sb[32 * g:32 * g + 4, c * CH:(c + 1) * CH],
                    start=True, stop=True, tile_position=(32 * g, 0))
                hid_slice[u] = (hid, g * 512)
            nc.scalar.activation(out=hid, in_=pre, func=AF.Tanh)

        def emit_dot(c):
            op = smallp.tile([1, 512], F32, tag=f"sm{c % 2}", name=f"dot{c}")
            for ht in range(4):
                hid, col = hid_slice[4 * c + ht]
                nc.tensor.matmul(
                    op, lhsT=ein["w"][:, ht:ht + 1],
                    rhs=hid[:, col:col + 512],
                    start=(ht == 0), stop=(ht == 3))
            orow = orowp.tile([1, 512], F32, tag="orow")
            nc.vector.tensor_scalar(
                out=orow, in0=op, scalar1=ein["c16"][0:1, 0:1],
                scalar2=float(OUT_SCALE), op0=ALU.add, op1=ALU.mult)
            nc.sync.dma_start(
                out=out_d[c * CH:(c + 1) * CH].rearrange(
                    "(o b) -> o b", o=1),
                in_=orow)

        with (
            tc.tile_pool(name="mainA", bufs=1, space="PSUM") as mainA,
            tc.tile_pool(name="mainB", bufs=1, space="PSUM") as mainB,
        ):
            next_chunk = 0
            chain_k = [99]
            for k in range(len(MAIN_SGS)):
                mfill(k)
                # finish the z matvecs, then the s/einsum/w chain, then dots
                if zcur[0] < TT:
                    emit_z(TT, 12)
                    if zcur[0] >= TT:
                        emit_chain()
                        chain_k[0] = k
                elif "w" not in ein:
                    if k >= chain_k[0] + 2:
                        emit_chain_b()
                elif k >= 2:
                    covered = sum(len(MAIN_SGS[i]) for i in range(k - 1))
                    nd = 0
                    cap = 2 if k < 14 else 3
                    while (next_chunk + 1) * 4 <= covered and nd < cap:
                        emit_dot(next_chunk)
                        next_chunk += 1
                        nd += 1
            while next_chunk < NCH:
                emit_dot(next_chunk)
                next_chunk += 1


def _get_nc():
    global _CACHED_NC
    if _CACHED_NC is None:
        _CACHED_NC = _build()
    return _CACHED_NC


def _prep_in_maps(inputs):
    f = lambda k: np.ascontiguousarray(np.asarray(inputs[k], np.float32))
    inputx = f("input")
    eq = float(np.asarray(inputs["eq_param"]).reshape(-1)[0])
    Wx1, bx1 = f("Wx1"), f("bx1")
    Wx2, bx2 = f("Wx2"), f("bx2")

    # main L1 operands replicated at 4 row strips
    wx1a = np.concatenate([Wx1, bx1[None, :]], axis=0)       # [4, 512]
    wx1a4 = np.tile(wx1a, (4, 1)).astype(np.float16)         # [16, 512]
    # wx2tb[df, b*512+it*128+i] = Wx2T[b*64+df, it*128+i]
    wx2tb = np.ascontiguousarray(
        Wx2.T.reshape(8, 64, 4, 128).transpose(1, 0, 2, 3).reshape(64, 4096)
    ).astype(np.float16)
    # bx2rb[df, b*16+m] = bx2[b*64+df]
    bx2rb = np.ascontiguousarray(
        np.repeat(bx2.reshape(8, 64).T[:, :, None], 16, axis=2).reshape(64, 128)
    ).astype(np.float16)

    # quad operands (shared across cores; quad fully replicated)
    qs, w1r, b1r = [], [], []
    wq2 = np.empty((HQ, 3 * HQ), np.float32)
    bq2r = np.empty((1, 3 * HQ), np.float32)
    for br, (qk, w1k, b1k, w2k, b2k) in enumerate([
            ("quad_x0", "Wq01", "bq01", "Wq02", "bq02"),
            ("quad_x1", "Wq11", "bq11", "Wq12", "bq12"),
            ("quad_x2", "Wq21", "bq21", "Wq22", "bq22")]):
        qs.append(f(qk)[:, 0])
        w1r.append(f(w1k)[0])
        b1r.append(f(b1k))
        wq2[:, br * HQ:(br + 1) * HQ] = f(w2k)
        bq2r[0, br * HQ:(br + 1) * HQ] = f(b2k)

    # qxa4[2g + r, S*512 + G*128 + i]: strip g holds tiles 12S+4g+G;
    # cols N.. carry the [W1; b1] weights for all 3 branches
    qxa4 = np.empty((6, N + 384), np.float32)
    qxc = np.empty((128, TT), np.float32)
    for t in range(TT):
        br, j = divmod(t, NT)
        nodes = qs[br][j * 128:(j + 1) * 128]
        S, g, G, _ = _qtile(t)
        qxa4[2 * g, S * 512 + G * 128: S * 512 + (G + 1) * 128] = nodes
        qxa4[2 * g + 1, 0:N] = 1.0
        qxc[:, t] = nodes * (np.pi * eq)
    for g in range(3):
        for br in range(3):
            qxa4[2 * g, N + 128 * br:N + 128 * (br + 1)] = w1r[br]
            qxa4[2 * g + 1, N + 128 * br:N + 128 * (br + 1)] = b1r[br]

    shared = dict(wx1a4=wx1a4, wx2tb=wx2tb, bx2rb=bx2rb,
                  qxa4=qxa4.astype(np.float16),
                  qxc=np.ascontiguousarray(qxc),
                  wq2=wq2.astype(np.float16), bq2r=bq2r.astype(np.float16))

    in_maps = []
    ones_row = np.ones((1, BL), np.float32)
    for c in range(NCORES):
        ish = inputx[c * BL:(c + 1) * BL]                    # [8192, 3]
        xTm = np.concatenate([ish.T, ones_row], axis=0)      # [4, 8192]
        m = dict(shared)
        m["xT1"] = np.ascontiguousarray(xTm).astype(np.float16)  # [4, 8192]
        in_maps.append(m)
    return in_maps


def _run(inputs, **kw):
    nc = _get_nc()
    in_maps = _prep_in_maps(inputs)
    res = run_bass_kernel_spmd(nc, in_maps, list(range(NCORES)), **kw)
    out = np.concatenate([res.results[c]["out"].reshape(-1)
                          for c in range(NCORES)]).astype(np.float32)
    return out, res


def kernel(**inputs) -> np.ndarray:
    out, _ = _run(inputs)
    return out


def kernel_traced(**inputs):
    """Correctness + NTFF profile (exec_time_ns) in one run."""
    return _run(inputs, trace=True)
